# revision 22
# baseline (speedup 1.0000x reference)
"""Trainium2 Bass kernel for nn_EntropyLoss (retrieval_knn).

Computes var([E(f1)-E(f0), E(f2)-E(f1)], ddof=1) where
E(f) = log(1 + sum_b sum_i r_ball[b, i]) and r_ball[b, i] is the K-th
nearest-neighbor distance (K = C//10 = 51, i.e. 52nd smallest including
the self-distance 0) among the C=512 channel vectors (dim H*W = 4096)
of sample b.

Data-parallel over the 48 (tensor, sample) units, 6 per core; per unit
the C x C "ranking" matrix m = G - sq_j/2 + 2048 (a per-row monotone
proxy for -d2/2) is built in PSUM and the per-row rank-52 value is
extracted.  v2 design (1.35x over the 249us pure max8/match_replace
baseline), all three compute engines in a per-unit software pipeline
PE(u) || bisect(u-1) || extract(u-2) inside a hardware For_i loop:

  PE    fp16 Gram with SYMMETRY: only diagonal+upper block-columns are
        computed directly (32 K=128-chunk matmuls into a full-width
        512-col PSUM bank seeded by a K=1 bias row); the strictly-lower
        blocks are transpose-mode matmuls of the (biased) fp32 SBUF
        copy of earlier row-blocks plus a K=1 per-partition
        compensation matmul (-b_I[p]).  ~35% fewer PE cycles; PE phase
        165us -> measured via ablation.
  ACT   copies each PSUM block to SBUF (fp32) and runs, for the 18
        i>0 blocks, a 5-step dyadic bisection for the rank-52
        threshold: count passes are Sign(m - t) activations with
        accum_out (720ns each), per-row threshold state NT [128,24]
        fp32 (host seeds t0 = mu + 1.2885 sigma per row, bracket +-40).
  GPSIM tiny per-iteration threshold updates (is_gt / mult+add /
        tensor_add on [128,3] tiles) -- keeping them off DVE avoids
        head-of-line blocking of DVE's FIFO by the bisection chain.
  DVE   classic 7-round max8/match_replace for the 6 i==0 blocks
        (rank 52 = col 3 of round 7), and for bisected blocks the
        final extraction: mask = max(sign,0); masked = m - 50000*mask
        (fp16); two max8 rounds + match_replace give ranks c+1..c+16
        which contain rank 52 since the final guarded count c (at
        t_fin = t_5 + 1.25 >= t*) satisfies 36 <= c <= 51 whp.
  host  picks column 51 - c, d2 = sq_i + 4096 - 2*m52,
        r = sqrt(max(d2, 0)), then the log/var tail in fp64.

Measured on HW (device-For_i loop slope n=100..500, 8 cores):
~184.5us/iteration (baseline 249us); rel err vs fp64 reference 7.1e-4
(gate 2e-2).  Ablations: PE+copies 165us, +bisect 178us.  FP8=1 env
flag enables fp8e4m3 directs (DoubleRow for I<2): PE phase 114us but
total only ~equal (selection-bound) at rel err 1.27e-2 -- not worth
the precision margin, left off.
"""
import sys

for _p in ("/opt/trn_rl_repo", "/root/.axon_site/_ro/trn_rl_repo"):
    if _p not in sys.path:
        sys.path.insert(0, _p)

import numpy as np

from concourse import bacc, mybir
from concourse.alu_op_type import AluOpType
from concourse.tile import TileContext
from concourse.bass_utils import run_bass_kernel_spmd

B, C, H, W = 16, 512, 64, 64
D = H * W  # 4096
K = C // 10  # 51 -> want 52nd smallest distance per row
RANK = K + 1  # 52
N_CORES = 8
N_TENSORS = 3
UNITS = N_TENSORS * B  # 48
UPC = UNITS // N_CORES  # units per core = 6
KCHUNKS = D // 128  # 32
RBLK = C // 128  # 4 row blocks per unit
NBLK = UPC * RBLK  # 24 blocks per core

# --- bisection parameters ---
N_ITERS = 5
DELTAS = [20.0, 10.0, 5.0, 2.5, 1.25]  # W0 = 40 bracket (t0 err ~6 rms)
GUARD = 1.25  # must be >= DELTAS[-1]
P_FULL = sum(DELTAS)  # 63.75
Z_QUANT = 1.2885  # Phi^-1(1 - 50.5/511)
NEGBIG = -60000.0
N_EXTRACT = 16  # two max8 rounds -> ranks c+1..c+16

N_CLS = 6  # units whose i==0 block stays classic max8 (rest fully bisected)
ROUNDS = RANK // 8 + (1 if RANK % 8 else 0)  # 7 classic rounds
SEL_COL = (RANK - 1) % 8  # rank-52 column within classic round 7
MASKSHIFT = -50000.0
SYM = True  # symmetric PE (transpose lower blocks)
import os
FP8 = os.environ.get("K_FP8", "0") == "1"  # fp8e4m3 directs (DR for I<2)
STAGGER = os.environ.get("K_STAG", "0") == "1"  # staggered For_i resets

TRACE = False  # test.py flips this for profiling
_LAST = {}  # debug stash

DMA_SPLIT = 4  # xt DMAs per sample




def _build_program(repeat=1, ablate=(), loop_n=None):
    """ablate: subset of {"sel", "ext", "mm", "dma"} for timing ablations."""
    nc = bacc.Bacc("TRN2", target_bir_lowering=False, debug=False)
    f16, f32 = mybir.dt.float16, mybir.dt.float32
    xdt = mybir.dt.float8e4 if FP8 else f16

    xt_d = nc.dram_tensor("xt", [UPC, 128, KCHUNKS * C], xdt, kind="ExternalInput")
    # sqn[s, j] = fp16(2048 - sq[s, j]/2) (per-column bias row)
    sqn_d = nc.dram_tensor("sqn", [UPC, C], f16, kind="ExternalInput")
    # nsq = -sqn (per-partition compensation for transposed blocks)
    nsq_d = nc.dram_tensor("nsq", [UPC, C], f16, kind="ExternalInput")
    # nt0[p, col] = -t0 for row (u, 128*I + p), col = _col_of(u, I)
    nt0_d = nc.dram_tensor("nt0", [128, NBLK], f32, kind="ExternalInput")
    ident_d = nc.dram_tensor("ident", [128, 128], f32, kind="ExternalInput")

    out_d = nc.dram_tensor("msel", [128, NBLK * N_EXTRACT], f16, kind="ExternalOutput")
    outc_d = nc.dram_tensor("mselc", [128, UPC * 8], f32, kind="ExternalOutput")
    cnt_d = nc.dram_tensor("cnt", [128, NBLK], f32, kind="ExternalOutput")

    kper = KCHUNKS // DMA_SPLIT  # k-chunks per DMA piece
    xt_view = xt_d.ap().rearrange("s p (d k c) -> s p d k c", d=DMA_SPLIT, k=kper)

    with TileContext(nc) as tc:
        with (
            tc.tile_pool(name="xpool", bufs=2 * DMA_SPLIT) as xpool,
            tc.tile_pool(name="consts", bufs=1) as consts,
            tc.tile_pool(name="msbp", bufs=20) as msbp,
            tc.tile_pool(name="state", bufs=2) as statep,
            tc.tile_pool(name="apool", bufs=3) as apool,
            tc.tile_pool(name="scrd", bufs=3) as scrd,
            tc.tile_pool(name="scra", bufs=3) as scra,
            tc.tile_pool(name="mskp", bufs=3) as mskp,
            tc.tile_pool(name="smalls", bufs=6) as smallp,
            tc.tile_pool(name="gps", bufs=6, space="PSUM") as gps,
        ):
            ones1 = consts.tile([1, 128], f16)
            nc.vector.memset(ones1, 1.0)
            c409 = consts.tile([128, 1], f32)
            nc.vector.memset(c409, 409.0)
            ident = consts.tile([128, 128], f32)
            nc.sync.dma_start(out=ident, in_=ident_d.ap())
            OUT = consts.tile([128, NBLK * N_EXTRACT], f16)
            OUTC = consts.tile([128, UPC * 8], f32)
            CNT = consts.tile([128, NBLK], f32)
            nc.vector.memset(CNT, 0.0)
            sqn_all = consts.tile([1, UPC * C], f16)
            nc.sync.dma_start(
                out=sqn_all, in_=sqn_d.ap().rearrange("s c -> (s c)").unsqueeze(0)
            )
            nsq_all = consts.tile([1, UPC * C], f16)
            nc.sync.dma_start(
                out=nsq_all, in_=nsq_d.ap().rearrange("s c -> (s c)").unsqueeze(0)
            )

            def pipeline_body(_iv=None):
                NT = statep.tile([128, NBLK], f32, tag="nt")
                nc.sync.dma_start(out=NT, in_=nt0_d.ap())
                NTv = NT.rearrange("p (u i) -> p u i", u=UPC)
                sign_f = mybir.ActivationFunctionType.Sign

                msb = [[None] * RBLK for _ in range(UPC)]
                xparts_cached = [None]

                def emit_unit_pe(s):
                    if "dma" in ablate and xparts_cached[0] is not None:
                        xparts = xparts_cached[0]
                    else:
                        xparts = []
                        for d in range(DMA_SPLIT):
                            xp = xpool.tile([128, kper, C], xdt, tag="xts")
                            nc.sync.dma_start(out=xp, in_=xt_view[s, :, d])
                            xparts.append(xp)
                        xparts_cached[0] = xparts

                    sqn = sqn_all[:, s * C : (s + 1) * C]
                    nsq = nsq_all[:, s * C : (s + 1) * C]

                    for I in range(RBLK):
                        g_ps = gps.tile([128, C], f32, tag="g")
                        nc.tensor.matmul(
                            out=g_ps, lhsT=ones1, rhs=sqn, start=True, stop=False
                        )
                        c0 = 128 * I if SYM else 0
                        nkc = 1 if "mm" in ablate else KCHUNKS
                        ntr = I if (SYM and "mm" not in ablate) else 0
                        use_dr = FP8 and I < 2 and nkc == KCHUNKS
                        if use_dr:
                            for kk in range(KCHUNKS // 2):
                                k2 = 2 * kk
                                xp = xparts[k2 // kper]
                                lo = k2 % kper
                                nc.tensor.matmul(
                                    out=g_ps[:, c0:C],
                                    lhsT=xp[:, lo : lo + 2, 128 * I : 128 * (I + 1)],
                                    rhs=xp[:, lo : lo + 2, c0:C],
                                    start=False,
                                    stop=(kk == KCHUNKS // 2 - 1 and ntr == 0),
                                    perf_mode=mybir.MatmulPerfMode.DoubleRow,
                                )
                        else:
                            for k in range(nkc):
                                xp = xparts[k // kper]
                                kk = k % kper
                                nc.tensor.matmul(
                                    out=g_ps[:, c0:C],
                                    lhsT=xp[:, kk, 128 * I : 128 * (I + 1)],
                                    rhs=xp[:, kk, c0:C],
                                    start=False,
                                    stop=(k == nkc - 1 and ntr == 0),
                                )
                        for J in range(ntr):
                            nc.tensor.matmul(
                                out=g_ps[:, 128 * J : 128 * (J + 1)],
                                lhsT=msb[s][J][:, 128 * I : 128 * (I + 1)],
                                rhs=ident,
                                is_transpose=True,
                                start=False,
                                stop=False,
                            )
                            nc.tensor.matmul(
                                out=g_ps[:, 128 * J : 128 * (J + 1)],
                                lhsT=nsq[:, 128 * I : 128 * (I + 1)],
                                rhs=ones1,
                                start=False,
                                stop=(J == ntr - 1),
                            )
                        m = msbp.tile([128, C], f32, tag="m")
                        nc.scalar.copy(out=m, in_=g_ps)
                        msb[s][I] = m

                ntfs = [None] * UPC

                def emit_unit_bisect(s):
                    if "sel" in ablate:
                        return
                    lo = 1 if s < N_CLS else 0
                    for it in range(N_ITERS):
                        A = apool.tile([128, RBLK], f32, tag="acc")
                        for I in range(lo, RBLK):
                            scr = scra.tile([128, C], f32, tag="scra")
                            nc.scalar.activation(
                                out=scr,
                                in_=msb[s][I],
                                func=sign_f,
                                bias=NTv[:, s, I : I + 1],
                                accum_out=A[:, I : I + 1],
                            )
                        # b = sign(A + 409) in {-1,+1} (A is even, 409 odd)
                        bt = smallp.tile([128, RBLK], f32, tag="b")
                        nc.scalar.activation(
                            out=bt[:, lo:RBLK], in_=A[:, lo:RBLK],
                            func=sign_f, bias=c409,
                        )
                        dlt = DELTAS[it]
                        ut = smallp.tile([128, RBLK], f32, tag="u")
                        nc.gpsimd.tensor_scalar_mul(
                            ut[:, lo:RBLK], bt[:, lo:RBLK], -dlt
                        )
                        nc.gpsimd.tensor_add(
                            NTv[:, s, lo:RBLK], NTv[:, s, lo:RBLK],
                            ut[:, lo:RBLK],
                        )

                    ntf = smallp.tile([128, RBLK], f32, tag="ntf")
                    nc.gpsimd.tensor_scalar(
                        out=ntf[:, lo:RBLK], in0=NTv[:, s, lo:RBLK],
                        scalar1=-GUARD, scalar2=None, op0=AluOpType.add,
                    )
                    ntfs[s] = ntf

                def emit_unit_extract(s):
                    if "sel" in ablate or "ext" in ablate:
                        return
                    # classic 7-round max8/match_replace for i == 0
                    if "cls" not in ablate and s < N_CLS:
                        m = msb[s][0]
                        mw = mskp.tile([128, C], f32, tag="mskc")
                        nc.vector.tensor_copy(mw, m)
                        for r in range(ROUNDS):
                            if r == ROUNDS - 1:
                                nc.vector.max(
                                    out=OUTC[:, s * 8 : s * 8 + 8], in_=mw
                                )
                            else:
                                mx = smallp.tile([128, 8], f32, tag="mx")
                                nc.vector.max(out=mx, in_=mw)
                                nc.vector.match_replace(
                                    out=mw, in_to_replace=mx, in_values=mw,
                                    imm_value=-1e30,
                                )

                    # final guarded pass + 2-round extraction:
                    # mask = max(sign(m - t_fin), 0); masked = m + MASKSHIFT*mask
                    ntf = ntfs[s]
                    lo = 1 if s < N_CLS else 0
                    for I in range(lo, RBLK):
                        col = s * RBLK + I
                        m = msb[s][I]
                        sc = scra.tile([128, C], f32, tag="scra")
                        nc.scalar.activation(
                            out=sc,
                            in_=m,
                            func=sign_f,
                            bias=ntf[:, I : I + 1],
                            accum_out=CNT[:, col : col + 1],
                        )
                        nc.vector.tensor_scalar_max(sc, sc, 0.0)
                        msk = mskp.tile([128, C], f16, tag="msk")
                        nc.vector.scalar_tensor_tensor(
                            out=msk, in0=sc, scalar=MASKSHIFT, in1=m,
                            op0=AluOpType.mult, op1=AluOpType.add,
                        )
                        o0 = col * N_EXTRACT
                        nc.vector.max(out=OUT[:, o0 : o0 + 8], in_=msk)
                        nc.vector.match_replace(
                            out=msk,
                            in_to_replace=OUT[:, o0 : o0 + 8],
                            in_values=msk,
                            imm_value=NEGBIG,
                        )
                        nc.vector.max(out=OUT[:, o0 + 8 : o0 + 16], in_=msk)

                # software pipeline: PE(u) || bisect(u-1) || extract(u-2)
                stag = STAGGER and loop_n is not None
                for s in range(UPC):
                    emit_unit_pe(s)
                    if s >= 2:
                        emit_unit_extract(s - 2)
                    if s >= 1:
                        emit_unit_bisect(s - 1)
                    if stag and s in (1, 3):
                        tc.stage_boundary()
                emit_unit_bisect(UPC - 1)
                if stag:
                    tc.stage_boundary()
                emit_unit_extract(UPC - 2)
                emit_unit_extract(UPC - 1)

                if "sel" not in ablate and "ext" not in ablate:
                    nc.sync.dma_start(out=out_d.ap(), in_=OUT)
                    nc.sync.dma_start(out=outc_d.ap(), in_=OUTC)
                    nc.sync.dma_start(out=cnt_d.ap(), in_=CNT)

            if loop_n is not None:
                with tc.For_i(0, loop_n, 1, staggered_reset=STAGGER) as _iv:
                    pipeline_body(_iv)
            else:
                for _rep in range(repeat):
                    pipeline_body()

    nc.compile()
    return nc


_PROGRAM = None


def _host_prep(feats):
    """feats: [UNITS, C, D] float32. Returns xt, sqn16, nsq16, nt0, sq64."""
    sq64 = np.einsum("ucd,ucd->uc", feats, feats, dtype=np.float64, casting="safe")
    sqn16 = (2048.0 - sq64 / 2.0).astype(np.float16)

    from concourse import mybir as _mb

    xdt_np = _mb.dt.np(_mb.dt.float8e4) if FP8 else np.float16
    xt = np.ascontiguousarray(
        feats.astype(xdt_np)
        .transpose(0, 2, 1)
        .reshape(UNITS, KCHUNKS, 128, C)
        .transpose(0, 2, 1, 3)
        .reshape(UNITS, 128, KCHUNKS * C)
    )

    # per-row threshold guess t0 = mu + z * sigma (bracket +-64 is ample)
    s32 = feats.sum(axis=1, dtype=np.float32)  # [U, D]
    dot = np.einsum("ucd,ud->uc", feats, s32)  # sum_j G_ij (incl self)
    Bv = sqn16.astype(np.float64)
    muG = (dot - sq64) / (C - 1)
    mub = (Bv.sum(axis=1, keepdims=True) - Bv) / (C - 1)
    varb = Bv.var(axis=1, keepdims=True)
    sigma = np.sqrt(sq64 + varb)
    t0 = muG + mub + Z_QUANT * sigma  # [U, C]
    nt0 = (-t0).astype(np.float32)
    return xt, sqn16, nt0, sq64


def _nt0_dev_layout(nt0_core):
    """nt0_core: [UPC, C] -> [128, NBLK], natural col = u*RBLK + i."""
    out = np.empty((128, NBLK), dtype=np.float32)
    for u in range(UPC):
        for i in range(RBLK):
            out[:, u * RBLK + i] = nt0_core[u, 128 * i : 128 * (i + 1)]
    return out


def kernel(feat0, feat1, feat2):
    global _PROGRAM
    feats = np.stack(
        [np.asarray(f).reshape(B, C, D) for f in (feat0, feat1, feat2)]
    ).reshape(UNITS, C, D)

    xt, sqn16, nt0, sq64 = _host_prep(feats)
    ident = np.eye(128, dtype=np.float32)

    if _PROGRAM is None:
        _PROGRAM = _build_program()
    nc = _PROGRAM
    in_maps = [
        {
            "xt": xt[c * UPC : (c + 1) * UPC],
            "sqn": sqn16[c * UPC : (c + 1) * UPC],
            "nsq": -sqn16[c * UPC : (c + 1) * UPC],
            "nt0": _nt0_dev_layout(nt0[c * UPC : (c + 1) * UPC]),
            "ident": ident,
        }
        for c in range(N_CORES)
    ]
    out = run_bass_kernel_spmd(
        nc, in_maps, core_ids=list(range(N_CORES)), trace=TRACE
    )
    _LAST.clear()
    _LAST["results"] = out

    m52 = np.empty((UNITS, C), dtype=np.float64)
    nclip = 0
    for core in range(N_CORES):
        sel = out.results[core]["msel"].astype(np.float64)  # [128, NBLK*16]
        selc = out.results[core]["mselc"].astype(np.float64)  # [128, UPC*8]
        araw = out.results[core]["cnt"].astype(np.float64)  # [128, NBLK]
        for u in range(UPC):
            for i in range(RBLK):
                if i == 0 and u < N_CLS:
                    vals = selc[:, u * 8 + SEL_COL]
                else:
                    col = u * RBLK + i
                    a = araw[:, col]
                    c = np.floor((a + C) / 2 + 0.25)
                    idx = np.rint(RANK - 1 - c).astype(np.int64)
                    bad = (idx < 0) | (idx >= N_EXTRACT)
                    nclip += int(bad.sum())
                    idx = np.clip(idx, 0, N_EXTRACT - 1)
                    vals = sel[np.arange(128), col * N_EXTRACT + idx]
                m52[core * UPC + u, 128 * i : 128 * (i + 1)] = vals
    _LAST["nclip"] = nclip
    if nclip:
        print(f"WARNING: {nclip} rows clipped rank index", file=sys.stderr)

    d2 = sq64 + 4096.0 - 2.0 * m52
    r = np.sqrt(np.clip(d2, 0.0, None))  # [UNITS, C]
    _LAST["r"] = r
    sums = r.reshape(N_TENSORS, B * C).sum(axis=1)
    e = np.log(sums + 1.0)
    deltas = np.array([e[1] - e[0], e[2] - e[1]])
    var = deltas.var(ddof=1)
    return np.asarray(var, dtype=np.float32)


# revision 23
# speedup vs baseline: 1.0262x; 1.0262x over previous
"""Trainium2 Bass kernel for nn_EntropyLoss (retrieval_knn).

Computes var([E(f1)-E(f0), E(f2)-E(f1)], ddof=1) where
E(f) = log(1 + sum_b sum_i r_ball[b, i]) and r_ball[b, i] is the K-th
nearest-neighbor distance (K = C//10 = 51, i.e. 52nd smallest including
the self-distance 0) among the C=512 channel vectors (dim H*W = 4096)
of sample b.

Data-parallel over the 48 (tensor, sample) units, 6 per core; per unit
the C x C "ranking" matrix m = G - sq_j/2 + 2048 (a per-row monotone
proxy for -d2/2) is built in PSUM and the per-row rank-52 value is
extracted.  v2 design (1.35x over the 249us pure max8/match_replace
baseline), all three compute engines in a per-unit software pipeline
PE(u) || bisect(u-1) || extract(u-2) inside a hardware For_i loop:

  PE    fp16 Gram with SYMMETRY: only diagonal+upper block-columns are
        computed directly (32 K=128-chunk matmuls into a full-width
        512-col PSUM bank seeded by a K=1 bias row); the strictly-lower
        blocks are transpose-mode matmuls of the (biased) fp32 SBUF
        copy of earlier row-blocks plus a K=1 per-partition
        compensation matmul (-b_I[p]).  ~35% fewer PE cycles; PE phase
        165us -> measured via ablation.
  ACT   copies each PSUM block to SBUF (fp32) and runs, for the 18
        i>0 blocks, a 5-step dyadic bisection for the rank-52
        threshold: count passes are Sign(m - t) activations with
        accum_out (720ns each), per-row threshold state NT [128,24]
        fp32 (host seeds t0 = mu + 1.2885 sigma per row, bracket +-40).
  GPSIM tiny per-iteration threshold updates (is_gt / mult+add /
        tensor_add on [128,3] tiles) -- keeping them off DVE avoids
        head-of-line blocking of DVE's FIFO by the bisection chain.
  DVE   classic 7-round max8/match_replace for the 6 i==0 blocks
        (rank 52 = col 3 of round 7), and for bisected blocks the
        final extraction: mask = max(sign,0); masked = m - 50000*mask
        (fp16); two max8 rounds + match_replace give ranks c+1..c+16
        which contain rank 52 since the final guarded count c (at
        t_fin = t_5 + 1.25 >= t*) satisfies 36 <= c <= 51 whp.
  host  picks column 51 - c, d2 = sq_i + 4096 - 2*m52,
        r = sqrt(max(d2, 0)), then the log/var tail in fp64.

Measured on HW (device-For_i loop slope n=100..500, 8 cores):
~184.5us/iteration (baseline 249us); rel err vs fp64 reference 7.1e-4
(gate 2e-2).  Ablations: PE+copies 165us, +bisect 178us.  FP8=1 env
flag enables fp8e4m3 directs (DoubleRow for I<2): PE phase 114us but
total only ~equal (selection-bound) at rel err 1.27e-2 -- not worth
the precision margin, left off.
"""
import sys

for _p in ("/opt/trn_rl_repo", "/root/.axon_site/_ro/trn_rl_repo"):
    if _p not in sys.path:
        sys.path.insert(0, _p)

import numpy as np

from concourse import bacc, mybir
from concourse.alu_op_type import AluOpType
from concourse.tile import TileContext
from concourse.bass_utils import run_bass_kernel_spmd

B, C, H, W = 16, 512, 64, 64
D = H * W  # 4096
K = C // 10  # 51 -> want 52nd smallest distance per row
RANK = K + 1  # 52
N_CORES = 8
N_TENSORS = 3
UNITS = N_TENSORS * B  # 48
UPC = UNITS // N_CORES  # units per core = 6
KCHUNKS = D // 128  # 32
RBLK = C // 128  # 4 row blocks per unit
NBLK = UPC * RBLK  # 24 blocks per core

# --- bisection parameters ---
N_ITERS = 5
DELTAS = [20.0, 10.0, 5.0, 2.5, 1.25]  # W0 = 40 bracket (t0 err ~6 rms)
GUARD = 1.25  # must be >= DELTAS[-1]
P_FULL = sum(DELTAS)  # 63.75
Z_QUANT = 1.2885  # Phi^-1(1 - 50.5/511)
NEGBIG = -60000.0
N_EXTRACT = 16  # two max8 rounds -> ranks c+1..c+16

N_CLS = 6  # units whose i==0 block stays classic max8 (rest fully bisected)
ROUNDS = RANK // 8 + (1 if RANK % 8 else 0)  # 7 classic rounds
SEL_COL = (RANK - 1) % 8  # rank-52 column within classic round 7
MASKSHIFT = -50000.0
SYM = True  # symmetric PE (transpose lower blocks)
import os
FP8 = os.environ.get("K_FP8", "0") == "1"  # fp8e4m3 directs (DR for I<2)
STAGGER = os.environ.get("K_STAG", "0") == "1"  # staggered For_i resets

TRACE = False  # test.py flips this for profiling
_LAST = {}  # debug stash

DMA_SPLIT = 4  # xt DMAs per sample




def _build_program(repeat=1, ablate=(), loop_n=None):
    """ablate: subset of {"sel", "ext", "mm", "dma"} for timing ablations."""
    nc = bacc.Bacc("TRN2", target_bir_lowering=False, debug=False)
    f16, f32 = mybir.dt.float16, mybir.dt.float32
    xdt = mybir.dt.float8e4 if FP8 else f16

    xt_d = nc.dram_tensor("xt", [UPC, 128, KCHUNKS * C], xdt, kind="ExternalInput")
    # sqn[s, j] = fp16(2048 - sq[s, j]/2) (per-column bias row)
    sqn_d = nc.dram_tensor("sqn", [UPC, C], f16, kind="ExternalInput")
    # nsq = -sqn (per-partition compensation for transposed blocks)
    nsq_d = nc.dram_tensor("nsq", [UPC, C], f16, kind="ExternalInput")
    # nt0[p, col] = -t0 for row (u, 128*I + p), col = _col_of(u, I)
    nt0_d = nc.dram_tensor("nt0", [128, NBLK], f32, kind="ExternalInput")
    ident_d = nc.dram_tensor("ident", [128, 128], f32, kind="ExternalInput")

    out_d = nc.dram_tensor("msel", [128, NBLK * N_EXTRACT], f16, kind="ExternalOutput")
    outc_d = nc.dram_tensor("mselc", [128, UPC * 8], f32, kind="ExternalOutput")
    cnt_d = nc.dram_tensor("cnt", [128, NBLK], f32, kind="ExternalOutput")

    kper = KCHUNKS // DMA_SPLIT  # k-chunks per DMA piece
    xt_view = xt_d.ap().rearrange("s p (d k c) -> s p d k c", d=DMA_SPLIT, k=kper)

    with TileContext(nc) as tc:
        with (
            tc.tile_pool(name="xpool", bufs=2 * DMA_SPLIT) as xpool,
            tc.tile_pool(name="consts", bufs=1) as consts,
            tc.tile_pool(name="msbp", bufs=20) as msbp,
            tc.tile_pool(name="state", bufs=2) as statep,
            tc.tile_pool(name="apool", bufs=3) as apool,
            tc.tile_pool(name="scrd", bufs=3) as scrd,
            tc.tile_pool(name="scra", bufs=3) as scra,
            tc.tile_pool(name="mskp", bufs=3) as mskp,
            tc.tile_pool(name="smalls", bufs=6) as smallp,
            tc.tile_pool(name="gps", bufs=6, space="PSUM") as gps,
        ):
            ones1 = consts.tile([1, 128], f16)
            nc.vector.memset(ones1, 1.0)
            ident = consts.tile([128, 128], f32)
            nc.sync.dma_start(out=ident, in_=ident_d.ap())
            OUT = consts.tile([128, NBLK * N_EXTRACT], f16)
            OUTC = consts.tile([128, UPC * 8], f32)
            CNT = consts.tile([128, NBLK], f32)
            nc.vector.memset(CNT, 0.0)
            sqn_all = consts.tile([1, UPC * C], f16)
            nc.sync.dma_start(
                out=sqn_all, in_=sqn_d.ap().rearrange("s c -> (s c)").unsqueeze(0)
            )
            nsq_all = consts.tile([1, UPC * C], f16)
            nc.sync.dma_start(
                out=nsq_all, in_=nsq_d.ap().rearrange("s c -> (s c)").unsqueeze(0)
            )

            def pipeline_body(_iv=None):
                NT = statep.tile([128, NBLK], f32, tag="nt")
                nc.sync.dma_start(out=NT, in_=nt0_d.ap())
                NTv = NT.rearrange("p (u i) -> p u i", u=UPC)
                sign_f = mybir.ActivationFunctionType.Sign

                msb = [[None] * RBLK for _ in range(UPC)]
                xparts_cached = [None]

                def emit_unit_pe(s):
                    if "dma" in ablate and xparts_cached[0] is not None:
                        xparts = xparts_cached[0]
                    else:
                        xparts = []
                        for d in range(DMA_SPLIT):
                            xp = xpool.tile([128, kper, C], xdt, tag="xts")
                            nc.sync.dma_start(out=xp, in_=xt_view[s, :, d])
                            xparts.append(xp)
                        xparts_cached[0] = xparts

                    sqn = sqn_all[:, s * C : (s + 1) * C]
                    nsq = nsq_all[:, s * C : (s + 1) * C]

                    for I in range(RBLK):
                        g_ps = gps.tile([128, C], f32, tag="g")
                        nc.tensor.matmul(
                            out=g_ps, lhsT=ones1, rhs=sqn, start=True, stop=False
                        )
                        c0 = 128 * I if SYM else 0
                        nkc = 1 if "mm" in ablate else KCHUNKS
                        ntr = I if (SYM and "mm" not in ablate) else 0
                        use_dr = FP8 and I < 2 and nkc == KCHUNKS
                        if use_dr:
                            for kk in range(KCHUNKS // 2):
                                k2 = 2 * kk
                                xp = xparts[k2 // kper]
                                lo = k2 % kper
                                nc.tensor.matmul(
                                    out=g_ps[:, c0:C],
                                    lhsT=xp[:, lo : lo + 2, 128 * I : 128 * (I + 1)],
                                    rhs=xp[:, lo : lo + 2, c0:C],
                                    start=False,
                                    stop=(kk == KCHUNKS // 2 - 1 and ntr == 0),
                                    perf_mode=mybir.MatmulPerfMode.DoubleRow,
                                )
                        else:
                            for k in range(nkc):
                                xp = xparts[k // kper]
                                kk = k % kper
                                nc.tensor.matmul(
                                    out=g_ps[:, c0:C],
                                    lhsT=xp[:, kk, 128 * I : 128 * (I + 1)],
                                    rhs=xp[:, kk, c0:C],
                                    start=False,
                                    stop=(k == nkc - 1 and ntr == 0),
                                )
                        for J in range(ntr):
                            nc.tensor.matmul(
                                out=g_ps[:, 128 * J : 128 * (J + 1)],
                                lhsT=msb[s][J][:, 128 * I : 128 * (I + 1)],
                                rhs=ident,
                                is_transpose=True,
                                start=False,
                                stop=False,
                            )
                            nc.tensor.matmul(
                                out=g_ps[:, 128 * J : 128 * (J + 1)],
                                lhsT=nsq[:, 128 * I : 128 * (I + 1)],
                                rhs=ones1,
                                start=False,
                                stop=(J == ntr - 1),
                            )
                        m = msbp.tile([128, C], f32, tag="m")
                        nc.scalar.copy(out=m, in_=g_ps)
                        msb[s][I] = m

                ntfs = [None] * UPC

                def emit_unit_bisect(s):
                    if "sel" in ablate:
                        return
                    lo = 1 if s < N_CLS else 0
                    for it in range(N_ITERS):
                        A = apool.tile([128, RBLK], f32, tag="acc")
                        for I in range(lo, RBLK):
                            scr = scra.tile([128, C], f32, tag="scra")
                            nc.scalar.activation(
                                out=scr,
                                in_=msb[s][I],
                                func=sign_f,
                                bias=NTv[:, s, I : I + 1],
                                accum_out=A[:, I : I + 1],
                            )
                        bt = smallp.tile([128, RBLK], f32, tag="b")
                        nc.gpsimd.tensor_scalar(
                            out=bt[:, lo:RBLK], in0=A[:, lo:RBLK],
                            scalar1=-409.0, scalar2=None,
                            op0=AluOpType.is_gt,
                        )
                        dlt = DELTAS[it]
                        ut = smallp.tile([128, RBLK], f32, tag="u")
                        nc.gpsimd.tensor_scalar(
                            out=ut[:, lo:RBLK], in0=bt[:, lo:RBLK],
                            scalar1=-2.0 * dlt, scalar2=dlt,
                            op0=AluOpType.mult, op1=AluOpType.add,
                        )
                        nc.gpsimd.tensor_add(
                            NTv[:, s, lo:RBLK], NTv[:, s, lo:RBLK],
                            ut[:, lo:RBLK],
                        )

                    ntf = smallp.tile([128, RBLK], f32, tag="ntf")
                    nc.gpsimd.tensor_scalar(
                        out=ntf[:, lo:RBLK], in0=NTv[:, s, lo:RBLK],
                        scalar1=-GUARD, scalar2=None, op0=AluOpType.add,
                    )
                    ntfs[s] = ntf

                def emit_unit_extract(s):
                    if "sel" in ablate or "ext" in ablate:
                        return
                    # classic 7-round max8/match_replace for i == 0
                    if "cls" not in ablate and s < N_CLS:
                        m = msb[s][0]
                        mw = mskp.tile([128, C], f32, tag="mskc")
                        nc.vector.tensor_copy(mw, m)
                        for r in range(ROUNDS):
                            if r == ROUNDS - 1:
                                nc.vector.max(
                                    out=OUTC[:, s * 8 : s * 8 + 8], in_=mw
                                )
                            else:
                                mx = smallp.tile([128, 8], f32, tag="mx")
                                nc.vector.max(out=mx, in_=mw)
                                nc.vector.match_replace(
                                    out=mw, in_to_replace=mx, in_values=mw,
                                    imm_value=-1e30,
                                )

                    # final guarded pass + 2-round extraction:
                    # mask = max(sign(m - t_fin), 0); masked = m + MASKSHIFT*mask
                    ntf = ntfs[s]
                    lo = 1 if s < N_CLS else 0
                    for I in range(lo, RBLK):
                        col = s * RBLK + I
                        m = msb[s][I]
                        sc = scra.tile([128, C], f32, tag="scra")
                        nc.scalar.activation(
                            out=sc,
                            in_=m,
                            func=sign_f,
                            bias=ntf[:, I : I + 1],
                            accum_out=CNT[:, col : col + 1],
                        )
                        nc.vector.tensor_scalar_max(sc, sc, 0.0)
                        msk = mskp.tile([128, C], f16, tag="msk")
                        nc.vector.scalar_tensor_tensor(
                            out=msk, in0=sc, scalar=MASKSHIFT, in1=m,
                            op0=AluOpType.mult, op1=AluOpType.add,
                        )
                        o0 = col * N_EXTRACT
                        nc.vector.max(out=OUT[:, o0 : o0 + 8], in_=msk)
                        nc.vector.match_replace(
                            out=msk,
                            in_to_replace=OUT[:, o0 : o0 + 8],
                            in_values=msk,
                            imm_value=NEGBIG,
                        )
                        nc.vector.max(out=OUT[:, o0 + 8 : o0 + 16], in_=msk)

                # software pipeline: PE(u) || bisect(u-1) || extract(u-2)
                stag = STAGGER and loop_n is not None
                for s in range(UPC):
                    emit_unit_pe(s)
                    if s >= 2:
                        emit_unit_extract(s - 2)
                    if s >= 1:
                        emit_unit_bisect(s - 1)
                    if stag and s in (1, 3):
                        tc.stage_boundary()
                emit_unit_bisect(UPC - 1)
                if stag:
                    tc.stage_boundary()
                emit_unit_extract(UPC - 2)
                emit_unit_extract(UPC - 1)

                if "sel" not in ablate and "ext" not in ablate:
                    nc.sync.dma_start(out=out_d.ap(), in_=OUT)
                    nc.sync.dma_start(out=outc_d.ap(), in_=OUTC)
                    nc.sync.dma_start(out=cnt_d.ap(), in_=CNT)

            if loop_n is not None:
                with tc.For_i(0, loop_n, 1, staggered_reset=STAGGER) as _iv:
                    pipeline_body(_iv)
            else:
                for _rep in range(repeat):
                    pipeline_body()

    nc.compile()
    return nc


_PROGRAM = None


def _host_prep(feats):
    """feats: [UNITS, C, D] float32. Returns xt, sqn16, nsq16, nt0, sq64."""
    sq64 = np.einsum("ucd,ucd->uc", feats, feats, dtype=np.float64, casting="safe")
    sqn16 = (2048.0 - sq64 / 2.0).astype(np.float16)

    from concourse import mybir as _mb

    xdt_np = _mb.dt.np(_mb.dt.float8e4) if FP8 else np.float16
    xt = np.ascontiguousarray(
        feats.astype(xdt_np)
        .transpose(0, 2, 1)
        .reshape(UNITS, KCHUNKS, 128, C)
        .transpose(0, 2, 1, 3)
        .reshape(UNITS, 128, KCHUNKS * C)
    )

    # per-row threshold guess t0 = mu + z * sigma (bracket +-64 is ample)
    s32 = feats.sum(axis=1, dtype=np.float32)  # [U, D]
    dot = np.einsum("ucd,ud->uc", feats, s32)  # sum_j G_ij (incl self)
    Bv = sqn16.astype(np.float64)
    muG = (dot - sq64) / (C - 1)
    mub = (Bv.sum(axis=1, keepdims=True) - Bv) / (C - 1)
    varb = Bv.var(axis=1, keepdims=True)
    sigma = np.sqrt(sq64 + varb)
    t0 = muG + mub + Z_QUANT * sigma  # [U, C]
    nt0 = (-t0).astype(np.float32)
    return xt, sqn16, nt0, sq64


def _nt0_dev_layout(nt0_core):
    """nt0_core: [UPC, C] -> [128, NBLK], natural col = u*RBLK + i."""
    out = np.empty((128, NBLK), dtype=np.float32)
    for u in range(UPC):
        for i in range(RBLK):
            out[:, u * RBLK + i] = nt0_core[u, 128 * i : 128 * (i + 1)]
    return out


def kernel(feat0, feat1, feat2):
    global _PROGRAM
    feats = np.stack(
        [np.asarray(f).reshape(B, C, D) for f in (feat0, feat1, feat2)]
    ).reshape(UNITS, C, D)

    xt, sqn16, nt0, sq64 = _host_prep(feats)
    ident = np.eye(128, dtype=np.float32)

    if _PROGRAM is None:
        _PROGRAM = _build_program()
    nc = _PROGRAM
    in_maps = [
        {
            "xt": xt[c * UPC : (c + 1) * UPC],
            "sqn": sqn16[c * UPC : (c + 1) * UPC],
            "nsq": -sqn16[c * UPC : (c + 1) * UPC],
            "nt0": _nt0_dev_layout(nt0[c * UPC : (c + 1) * UPC]),
            "ident": ident,
        }
        for c in range(N_CORES)
    ]
    out = run_bass_kernel_spmd(
        nc, in_maps, core_ids=list(range(N_CORES)), trace=TRACE
    )
    _LAST.clear()
    _LAST["results"] = out

    m52 = np.empty((UNITS, C), dtype=np.float64)
    nclip = 0
    for core in range(N_CORES):
        sel = out.results[core]["msel"].astype(np.float64)  # [128, NBLK*16]
        selc = out.results[core]["mselc"].astype(np.float64)  # [128, UPC*8]
        araw = out.results[core]["cnt"].astype(np.float64)  # [128, NBLK]
        for u in range(UPC):
            for i in range(RBLK):
                if i == 0 and u < N_CLS:
                    vals = selc[:, u * 8 + SEL_COL]
                else:
                    col = u * RBLK + i
                    a = araw[:, col]
                    c = np.floor((a + C) / 2 + 0.25)
                    idx = np.rint(RANK - 1 - c).astype(np.int64)
                    bad = (idx < 0) | (idx >= N_EXTRACT)
                    nclip += int(bad.sum())
                    idx = np.clip(idx, 0, N_EXTRACT - 1)
                    vals = sel[np.arange(128), col * N_EXTRACT + idx]
                m52[core * UPC + u, 128 * i : 128 * (i + 1)] = vals
    _LAST["nclip"] = nclip
    if nclip:
        print(f"WARNING: {nclip} rows clipped rank index", file=sys.stderr)

    d2 = sq64 + 4096.0 - 2.0 * m52
    r = np.sqrt(np.clip(d2, 0.0, None))  # [UNITS, C]
    _LAST["r"] = r
    sums = r.reshape(N_TENSORS, B * C).sum(axis=1)
    e = np.log(sums + 1.0)
    deltas = np.array([e[1] - e[0], e[2] - e[1]])
    var = deltas.var(ddof=1)
    return np.asarray(var, dtype=np.float32)
